# revision 16
# baseline (speedup 1.0000x reference)
"""BatchSRU Trainium2 kernel (nn_BatchSRU_27556510171508) — v4.

Full inputs: x (2048, 8, 128, 16) f32, W (16, 128, 384), b (16, 256).
Sharding: data-parallel over the inner batch B=8 -> one batch row per
NeuronCore (zero cross-core communication); W/b replicated.

v4 over v3 (engine rebalance; v3 sim: DVE 167us busy = bottleneck):
  - t = c - x^T moved from DVE to GpSimd (idle; ~1.1us/op there but off
    the DVE roofline)
  - carry save ops eliminated: each chunk's scan chains its initial
    state directly to the previous chunk's cw[:, j, -1:] (cw gets
    per-group tags so the previous chunk's tile is still alive)
  - the two per-instance sigmoids fused into ONE ACT instruction over
    [128, 2*LC]: gate biases are pre-added into the f/r PSUM banks by
    rank-1 (K=1) matmuls (bias row x ones), so both gates share bias=0
  - u = r*t written onto the g' tile (dead after the scan) to keep
    SBUF within budget
"""

import numpy as np
from contextlib import ExitStack

import concourse.bacc as bacc
import concourse.tile as tile
from concourse import mybir
from concourse.masks import make_identity

F32 = mybir.dt.float32
BF16 = mybir.dt.bfloat16
AL = mybir.AluOpType
AF = mybir.ActivationFunctionType

L, B, D, NB = 2048, 8, 128, 16
LC = 512                 # l-chunk
NCH = L // LC            # 4 chunks
QNB = 4                  # instances per scan group
NQ = NB // QNB           # 4 groups
NLS = LC // 128          # 4 l-subtiles per chunk

N_CORES = 8


def _build(repeat: int = 1, *, sig_pair=True, t_pool=True, u_pool=False,
           scan_pool=False, unroll=False, lag=6, ulag=2, blag=1):
    nc = bacc.Bacc("TRN2")
    x = nc.dram_tensor("x", [L, NB, D], F32, kind="ExternalInput")
    w = nc.dram_tensor("w", [NB, D, 3 * D], F32, kind="ExternalInput")
    bb = nc.dram_tensor("bb", [NB, 2 * D], F32, kind="ExternalInput")
    out = nc.dram_tensor("out", [L, NB, D], F32, kind="ExternalOutput")

    with tile.TileContext(nc) as tc, ExitStack() as ctx:
        const = ctx.enter_context(tc.tile_pool(name="const", bufs=1))

        ident = const.tile([128, 128], F32)
        make_identity(nc, ident)
        identb = const.tile([128, 128], BF16)
        make_identity(nc, identb)
        wr = const.tile([128, NB, 3 * D], BF16)
        carry0 = const.tile([128, NB], BF16)
        nc.vector.memset(carry0, 0.0)

        if sig_pair:
            # bias rows for the rank-1 psum prefill matmuls: all on
            # partition 0 (stationary base partition must be 0/32/64);
            # the one-time f32->bf16 copy hides under the warmup DMA
            browf = const.tile([1, 2 * NB, 128], F32)
            nc.scalar.dma_start(
                out=browf, in_=bb.rearrange("n (g d) -> (n g d)", g=2)[None]
            )
            brow = const.tile([1, 2 * NB, 128], BF16)
            nc.vector.tensor_copy(brow, browf)
            ones_t = const.tile([1, LC], BF16)
            nc.gpsimd.memset(ones_t, 1.0)
        else:
            bsb = const.tile([128, NB, 2], F32)
            nc.scalar.dma_start(
                out=bsb, in_=bb.rearrange("n (g d) -> d n g", d=128)
            )

        # W: DMA as f32 then round to bf16. Four pieces so the x-chunk
        # loads interleave with them; copies on DVE (idle during warmup).
        with tc.tile_pool(name="wtmp_pool", bufs=2) as wtmp_pool:
            for wi in range(4):
                sl = slice(wi * 4, (wi + 1) * 4)
                wtmp = wtmp_pool.tile([128, 4, 3 * D], F32, tag="wtmp")
                nc.scalar.dma_start(out=wtmp, in_=w.transpose([1, 0, 2])[:, sl])
                nc.vector.tensor_copy(wr[:, sl], wtmp)

        xpool = ctx.enter_context(tc.tile_pool(name="xpool", bufs=2))
        sb = ctx.enter_context(tc.tile_pool(name="sb", bufs=2))
        # x^T tiles double as the u = r*t result (written once the gates
        # are consumed), so their ring runs one deeper
        xtp = ctx.enter_context(tc.tile_pool(name="xtp", bufs=3))
        # PSUM: x-transpose/x_tilde bank on a ring of 3 (its WAR reader,
        # g', sits late in the chain and was the binding pipeline cycle),
        # f/r pair on a ring of 2, out-transpose staging single-buffered
        px = ctx.enter_context(tc.tile_pool(name="px", bufs=3, space="PSUM"))
        pfr = ctx.enter_context(tc.tile_pool(name="pfr", bufs=2, space="PSUM"))
        ph = ctx.enter_context(tc.tile_pool(name="ph", bufs=1, space="PSUM"))

        import contextlib

        # back pieces (out-transpose + h-add + final out-DMA) are deferred
        # across group AND chunk boundaries via this queue so no engine
        # ever waits on the scan->highway tail of the current group
        # piece state: [ready_gidx, uw, qq, ls, xts_tile, lc, hps]
        pending = []
        u_pend = []  # (ready_gidx, gw_tile, r_src, t_src) — group u = r*t
        dma_left = {}  # id(xts_tile) -> (remaining piece count, lc, ls)

        def piece_transposes(p):
            # out-transpose u = r*(c-x) for one l-subtile
            ready, uw, qq, ls, xts_t, plc, _ = p
            hps = ph.tile([128, QNB * 128], BF16, tag="ph", name="hps")
            for j in range(QNB):
                nc.tensor.transpose(
                    hps[:, j * 128 : (j + 1) * 128],
                    uw[:, j, ls * 128 : ls * 128 + 128],
                    identb,
                )
            p[6] = hps

        def piece_back(p):
            # fuse h = u^T + x onto the x tile; after the last group's
            # piece for this subtile, emit its output DMA
            ready, uw, qq, ls, xts_t, plc, hps = p
            xv = xts_t[:, qq * QNB * D : (qq + 1) * QNB * D]
            nc.vector.tensor_tensor(xv, hps, xv, AL.add)
            left, dlc, dls = dma_left[id(xts_t)]
            left -= 1
            dma_left[id(xts_t)] = (left, dlc, dls)
            if left == 0:
                l0 = dlc * LC + dls * 128
                nc.sync.dma_start(
                    out=out[l0 : l0 + 128].rearrange("l n d -> l (n d)"),
                    in_=xts_t,
                )

        def drain_pending(gidx):
            if pending and gidx >= pending[0][0] + lag:
                p = pending.pop(0)
                piece_transposes(p)
                piece_back(p)

        def drain_u(gidx):
            while u_pend and gidx >= u_pend[0][0]:
                _, uxt, r_src, t_src = u_pend.pop(0)
                # u = r * t for the whole group in one op (t sits in the
                # dead f slots; the result overwrites the dead x^T tile,
                # whose ring predecessors have only early readers)
                nc.vector.tensor_tensor(uxt[:, :], r_src, t_src, AL.mult)

        cw_prev = {}  # q -> previous chunk's cw tile (for scan carry chaining)

        n_unroll = repeat if unroll else 1
        loop_cm = (
            tc.For_i(0, repeat)
            if repeat > 1 and not unroll
            else contextlib.nullcontext()
        )
        with loop_cm:
         for lc0 in range(NCH * n_unroll):
            lc = lc0 % NCH
            xts = []
            for ls in range(NLS):
                xt_in = xpool.tile([128, D * NB], F32, tag=f"X{ls}")
                l0 = lc * LC + ls * 128
                nc.sync.dma_start(
                    out=xt_in, in_=x[l0 : l0 + 128].rearrange("l n d -> l (n d)")
                )
                xts.append(xt_in)
                dma_left[id(xt_in)] = (NQ, lc, ls)

            # per-group SBUF tiles, 2 groups in flight (c gets per-group
            # tags so the previous chunk's carry column stays alive)
            def gtiles(q):
                s = q % 2
                xTw = xtp.tile([128, QNB, LC], BF16, tag=f"xT{s}", name=f"xT{s}")
                if sig_pair:
                    frw = sb.tile(
                        [128, QNB, 2, LC], BF16, tag=f"fr{s}", name=f"fr{s}"
                    )
                else:
                    fw = sb.tile([128, QNB, LC], BF16, tag=f"f{s}", name=f"f{s}")
                    rw = sb.tile([128, QNB, LC], BF16, tag=f"r{s}", name=f"r{s}")
                    frw = (fw, rw)
                gw = sb.tile([128, QNB, LC], BF16, tag=f"g{s}", name=f"g{s}")
                cw = sb.tile([128, QNB, LC], BF16, tag=f"c{q}", name=f"c{q}")
                return xTw, frw, gw, cw

            def bias_mms(i, pfr_i):
                if not sig_pair:
                    return
                # rank-1 prefill: f bank = bf[d] broadcast, r bank = br
                nc.tensor.matmul(
                    pfr_i[:, 0], brow[0:1, 2 * i], ones_t[0:1, :],
                    start=True, stop=False,
                )
                nc.tensor.matmul(
                    pfr_i[:, 1], brow[0:1, 2 * i + 1], ones_t[0:1, :],
                    start=True, stop=False,
                )

            def in_transpose_pe(i, px_i):
                # 4 l-subtiles of instance nb -> the px psum bank
                for ls in range(NLS):
                    xg = xts[ls][:, i * D : (i + 1) * D]
                    nc.tensor.transpose(
                        px_i[:, ls * 128 : (ls + 1) * 128], xg, ident
                    )

            def in_transpose_copy(i, px_i, xTw):
                # rounding copy psum -> bf16 SBUF (ACT); issued after the
                # current instance's sigmoid so it never delays it
                nc.scalar.copy(xTw[:, i % QNB], px_i)

            grp = {}  # q -> group SBUF tiles

            pu_i = [None] * (NB + 1)

            def alloc_pu(i):
                px_i = px.tile([128, LC], F32, tag="px", name="px")
                pfr_i = pfr.tile([128, 2, LC], F32, tag="pfr", name="pfr")
                pu_i[i] = (px_i, pfr_i)

            # prime: bias prefill + in-transpose for instance 0
            grp[0] = gtiles(0)
            alloc_pu(0)
            bias_mms(0, pu_i[0][1])
            in_transpose_pe(0, pu_i[0][0])
            in_transpose_copy(0, pu_i[0][0], grp[0][0])

            for i in range(NB):
                q, j = i // QNB, i % QNB
                gidx = lc * NB + i
                xTw, frw, gw, cw = grp[q]

                # next instance's PE transposes ahead of this one's
                # matmuls (the ACT copy is issued after this instance's
                # sigmoid)
                if i + 1 < NB:
                    qn = (i + 1) // QNB
                    if (i + 1) % QNB == 0:
                        grp[qn] = gtiles(qn)
                    alloc_pu(i + 1)
                    bias_mms(i + 1, pu_i[i + 1][1])
                    in_transpose_pe(i + 1, pu_i[i + 1][0])

                px_i, pfr_i = pu_i[i]
                # gate matmuls first (they feed the sigmoid, the longest
                # chain); x_tilde then overwrites the transpose bank
                if sig_pair:
                    nc.tensor.matmul(
                        pfr_i[:, 0], wr[:, i, 128:256], xTw[:, j],
                        start=False, stop=True,
                    )
                    nc.tensor.matmul(
                        pfr_i[:, 1], wr[:, i, 256:384], xTw[:, j],
                        start=False, stop=True,
                    )
                    nc.tensor.matmul(
                        px_i, wr[:, i, 0:128], xTw[:, j], start=True, stop=True
                    )
                    # both gates in one ACT op (bias already in psum)
                    nc.scalar.activation(
                        frw[:, j], pfr_i[:, :], AF.Sigmoid, scale=1.0
                    )
                    f_ap, r_ap = frw[:, j, 0], frw[:, j, 1]
                else:
                    nc.tensor.matmul(
                        pfr_i[:, 0], wr[:, i, 128:256], xTw[:, j],
                        start=True, stop=True,
                    )
                    nc.tensor.matmul(
                        pfr_i[:, 1], wr[:, i, 256:384], xTw[:, j],
                        start=True, stop=True,
                    )
                    nc.tensor.matmul(
                        px_i, wr[:, i, 0:128], xTw[:, j], start=True, stop=True
                    )
                    fw, rw = frw
                    nc.scalar.activation(
                        fw[:, j], pfr_i[:, 0], AF.Sigmoid,
                        bias=bsb[:, i, 0:1], scale=1.0,
                    )
                    nc.scalar.activation(
                        rw[:, j], pfr_i[:, 1], AF.Sigmoid,
                        bias=bsb[:, i, 1:2], scale=1.0,
                    )
                    f_ap, r_ap = fw[:, j], rw[:, j]

                if i + 1 < NB:
                    in_transpose_copy(
                        i + 1, pu_i[i + 1][0], grp[(i + 1) // QNB][0]
                    )

                # g' = (f - 1) * x_tilde  (DVE, fused)
                nc.vector.scalar_tensor_tensor(
                    gw[:, j], f_ap, -1.0, px_i, AL.add, AL.mult
                )
                # per-instance scan: state = f*state - g'  (fp32 state).
                # initial chains straight to the previous chunk's last
                # column (cw tags are per-group so it is still alive).
                init = (
                    carry0[:, i : i + 1]
                    if lc == 0
                    else cw_prev[q][:, j, LC - 1 : LC]
                )
                scan_eng = nc.gpsimd if scan_pool else nc.vector
                scan_eng.tensor_tensor_scan(
                    cw[:, j], f_ap, gw[:, j], init,
                    op0=AL.mult, op1=AL.subtract,
                )
                # highway: t = c - x^T (GpSimd) written onto the dead
                # f slot; u = r*t deferred as one whole-group DVE op so it
                # never blocks the next scan behind the Pool round-trip
                t_eng = nc.gpsimd if t_pool else nc.vector
                t_eng.tensor_tensor(f_ap, cw[:, j], xTw[:, j], AL.subtract)
                if j == QNB - 1:
                    cw_prev[q] = cw
                    if sig_pair:
                        u_pend.append((gidx + ulag, xTw, frw[:, :, 1], frw[:, :, 0]))
                    else:
                        u_pend.append((gidx + ulag, xTw, frw[1][:, :], frw[0][:, :]))
                    for ls in range(NLS):
                        pending.append([gidx, xTw, q, ls, xts[ls], lc, None])
                drain_u(gidx)
                drain_pending(gidx)

         # flush remaining group-u ops, back pieces + final out-DMAs
         drain_u(1 << 30)
         while pending:
            drain_pending(1 << 30)
            drain_pending(1 << 30)

    nc.finalize()
    return nc


_NC_CACHE = None


def _get_nc():
    global _NC_CACHE
    if _NC_CACHE is None:
        _NC_CACHE = _build()
    return _NC_CACHE


def make_in_maps(x, W, b):
    # per-core layout (L, NB, D): every on-device access pattern is then
    # contiguous (strided PE moving-operand reads are ~4x slower on HW)
    return [
        dict(x=np.ascontiguousarray(x[:, i].transpose(0, 2, 1)), w=W, bb=b)
        for i in range(N_CORES)
    ]


def assemble(outs):
    # outs: per-core (L, NB, D) -> full (L, B, D, NB)
    return np.stack([o.transpose(0, 2, 1) for o in outs], axis=1)


def kernel(x: np.ndarray, W: np.ndarray, b: np.ndarray) -> np.ndarray:
    assert x.shape == (L, B, D, NB) and W.shape == (NB, D, 3 * D)
    from concourse.bass_utils import run_bass_kernel_spmd

    nc = _get_nc()
    x = np.asarray(x, dtype=np.float32)
    W = np.asarray(W, dtype=np.float32)
    b = np.asarray(b, dtype=np.float32)
    in_maps = make_in_maps(x, W, b)
    results = run_bass_kernel_spmd(nc, in_maps, core_ids=list(range(N_CORES))).results
    return assemble([results[i]["out"] for i in range(N_CORES)])


# revision 20
# speedup vs baseline: 1.2091x; 1.2091x over previous
"""BatchSRU Trainium2 kernel (nn_BatchSRU_27556510171508) — v5.

Full inputs: x (2048, 8, 128, 16) f32, W (16, 128, 384), b (16, 256).
Sharding: data-parallel over the inner batch B=8 -> one batch row per
NeuronCore (zero cross-core communication); W/b replicated.

v5 = v3 pipeline (all elementwise work on DVE; GpSimd measured ~3x
slower on HW and poisons the scan->highway chain) plus:
  - t = c - x^T computed as instance PAIRS ([128, 2*LC] ops) right
    after the pair's second scan — halves the op count and bubbles
  - u = r*t computed as ONE whole-group op [128, 4*LC], deferred two
    instances so it never sits between two scans on the in-order DVE
  - per-instance carry-save ACT copies eliminated: each chunk's scan
    chains initial= straight to the previous chunk's cw[:, j, -1:]
    (cw tags are per-group so the previous chunk's tile stays alive)
"""
import numpy as np
from contextlib import ExitStack

import concourse.bacc as bacc
import concourse.tile as tile
from concourse import mybir
from concourse.masks import make_identity

F32 = mybir.dt.float32
BF16 = mybir.dt.bfloat16
AL = mybir.AluOpType
AF = mybir.ActivationFunctionType

L, B, D, NB = 2048, 8, 128, 16
LC = 512
NCH = L // LC
QNB = 4
NQ = NB // QNB
NLS = LC // 128

N_CORES = 8


def _build(repeat: int = 1, unroll=False, carry_elim=True, ugroup=True, tpair=True, psplit=False, u_pool=False, t_pool=False):
    nc = bacc.Bacc("TRN2")
    x = nc.dram_tensor("x", [L, NB, D], F32, kind="ExternalInput")
    w = nc.dram_tensor("w", [NB, D, 3 * D], F32, kind="ExternalInput")
    bb = nc.dram_tensor("bb", [NB, 2 * D], F32, kind="ExternalInput")
    out = nc.dram_tensor("out", [L, NB, D], F32, kind="ExternalOutput")

    with tile.TileContext(nc) as tc, ExitStack() as ctx:
        const = ctx.enter_context(tc.tile_pool(name="const", bufs=1))

        ident = const.tile([128, 128], F32)
        make_identity(nc, ident)
        identb = const.tile([128, 128], BF16)
        make_identity(nc, identb)
        wr = const.tile([128, NB, 3 * D], BF16)
        bsb = const.tile([128, NB, 2], F32)
        nc.scalar.dma_start(out=bsb, in_=bb.rearrange("n (g d) -> d n g", d=128))
        carry = const.tile([128, NB], BF16)
        nc.vector.memset(carry, 0.0)

        with tc.tile_pool(name="wtmp_pool", bufs=1) as wtmp_pool:
            wtmp = wtmp_pool.tile([128, NB, 3 * D], F32)
            for wi in range(4):
                sl = slice(wi * 4, (wi + 1) * 4)
                nc.scalar.dma_start(out=wtmp[:, sl], in_=w.transpose([1, 0, 2])[:, sl])
                nc.vector.tensor_copy(wr[:, sl], wtmp[:, sl])

        xpool = ctx.enter_context(tc.tile_pool(name="xpool", bufs=2))
        sb = ctx.enter_context(tc.tile_pool(name="sb", bufs=2))
        if psplit:
            px = ctx.enter_context(tc.tile_pool(name="px", bufs=3, space="PSUM"))
            pfr = ctx.enter_context(tc.tile_pool(name="pfr", bufs=2, space="PSUM"))
            ph = ctx.enter_context(tc.tile_pool(name="ph", bufs=1, space="PSUM"))
        else:
            pu = ctx.enter_context(tc.tile_pool(name="pu", bufs=2, space="PSUM"))
            ph = ctx.enter_context(tc.tile_pool(name="ph", bufs=2, space="PSUM"))

        import contextlib

        pending = []
        u_pend = []
        dma_left = {}

        def emit_back_piece(rw, qq, ls, xts_t, plc):
            hps = ph.tile([128, QNB * 128], BF16, tag="ph", name="hps")
            for j in range(QNB):
                nc.tensor.transpose(
                    hps[:, j * 128 : (j + 1) * 128],
                    rw[:, j, ls * 128 : ls * 128 + 128],
                    identb,
                )
            xv = xts_t[:, qq * QNB * D : (qq + 1) * QNB * D]
            nc.vector.tensor_tensor(xv, hps, xv, AL.add)
            left, dlc, dls = dma_left[id(xts_t)]
            left -= 1
            dma_left[id(xts_t)] = (left, dlc, dls)
            if left == 0:
                l0 = dlc * LC + dls * 128
                nc.sync.dma_start(
                    out=out[l0 : l0 + 128].rearrange("l n d -> l (n d)"),
                    in_=xts_t,
                )

        def drain_pending(gidx, lag=6):
            if pending and gidx >= pending[0][0] + lag:
                ready, rw, qq, ls, xts_t, plc = pending.pop(0)
                emit_back_piece(rw, qq, ls, xts_t, plc)

        def drain_u(gidx):
            while u_pend and gidx >= u_pend[0][0]:
                _, uw_, rw_, tw_ = u_pend.pop(0)
                eng = nc.gpsimd if u_pool else nc.vector
                eng.tensor_tensor(uw_[:, :], rw_[:, :], tw_[:, :], AL.mult)

        cw_prev = {}
        n_unroll = repeat if unroll else 1
        loop_cm = (
            tc.For_i(0, repeat) if repeat > 1 and not unroll
            else contextlib.nullcontext()
        )
        with loop_cm:
         for lc0 in range(NCH * n_unroll):
            lc = lc0 % NCH
            xts = []
            for ls in range(NLS):
                xt_in = xpool.tile([128, D * NB], F32, tag=f"X{ls}")
                l0 = lc * LC + ls * 128
                nc.sync.dma_start(
                    out=xt_in, in_=x[l0 : l0 + 128].rearrange("l n d -> l (n d)")
                )
                xts.append(xt_in)
                dma_left[id(xt_in)] = (NQ, lc, ls)

            def gtiles(q):
                s = q % 2
                xTw = sb.tile([128, QNB, LC], BF16, tag=f"xT{s}", name=f"xT{s}")
                fw = sb.tile([128, QNB, LC], BF16, tag=f"f{s}", name=f"f{s}")
                rw = sb.tile([128, QNB, LC], BF16, tag=f"r{s}", name=f"r{s}")
                gw = sb.tile([128, QNB, LC], BF16, tag=f"g{s}", name=f"g{s}")
                cw = sb.tile(
                    [128, QNB, LC], BF16,
                    tag=(f"c{q}" if carry_elim else f"c{s}"), name="cw",
                )
                tw = sb.tile([128, QNB, LC], BF16, tag=f"t{s}", name=f"t{s}")
                uw = sb.tile([128, QNB, LC], BF16, tag=f"u{s}", name=f"u{s}")
                return xTw, fw, rw, gw, cw, tw, uw

            def in_transpose(i, pui, xTw):
                j = i % QNB
                px_i = pui[0]
                for ls in range(NLS):
                    xg = xts[ls][:, i * D : (i + 1) * D]
                    nc.tensor.transpose(
                        px_i[:, ls * 128 : (ls + 1) * 128], xg, ident
                    )
                nc.scalar.copy(xTw[:, j], px_i)

            grp = {}
            pu_i = [None] * (NB + 1)

            def alloc_pu():
                if psplit:
                    return (
                        px.tile([128, LC], F32, tag="px", name="px"),
                        pfr.tile([128, 2, LC], F32, tag="pfr", name="pfr"),
                    )
                t_ = pu.tile([128, 3, LC], F32, tag="pu", name="pu")
                return (t_[:, 0], t_[:, 1:3])

            grp[0] = gtiles(0)
            pu_i[0] = alloc_pu()
            in_transpose(0, pu_i[0], grp[0][0])

            for i in range(NB):
                q, j = i // QNB, i % QNB
                gidx = lc0 * NB + i
                xTw, fw, rw, gw, cw, tw, uw = grp[q]

                if i + 1 < NB:
                    qn = (i + 1) // QNB
                    if (i + 1) % QNB == 0:
                        grp[qn] = gtiles(qn)
                    pu_i[i + 1] = alloc_pu()
                    in_transpose(i + 1, pu_i[i + 1], grp[qn][0])

                px_i, pfr_i = pu_i[i]
                nc.tensor.matmul(
                    pfr_i[:, 0], wr[:, i, 128:256], xTw[:, j], start=True, stop=True
                )
                nc.tensor.matmul(
                    pfr_i[:, 1], wr[:, i, 256:384], xTw[:, j], start=True, stop=True
                )
                nc.tensor.matmul(
                    px_i, wr[:, i, 0:128], xTw[:, j], start=True, stop=True
                )
                nc.scalar.activation(
                    fw[:, j], pfr_i[:, 0], AF.Sigmoid, bias=bsb[:, i, 0:1], scale=1.0
                )
                nc.scalar.activation(
                    rw[:, j], pfr_i[:, 1], AF.Sigmoid, bias=bsb[:, i, 1:2], scale=1.0
                )
                nc.vector.scalar_tensor_tensor(
                    gw[:, j], fw[:, j], -1.0, px_i, AL.add, AL.mult
                )
                if carry_elim:
                    init = (
                        carry[:, i : i + 1]
                        if lc == 0
                        else cw_prev[q][:, j, LC - 1 : LC]
                    )
                else:
                    init = carry[:, i : i + 1]
                nc.vector.tensor_tensor_scan(
                    cw[:, j], fw[:, j], gw[:, j], init,
                    op0=AL.mult, op1=AL.subtract,
                )
                if not carry_elim:
                    nc.scalar.copy(carry[:, i : i + 1], cw[:, j, LC - 1 : LC])
                if t_pool:
                    nc.gpsimd.tensor_tensor(
                        tw[:, j], cw[:, j], xTw[:, j], AL.subtract
                    )
                elif tpair:
                    if j % 2 == 1:
                        nc.vector.tensor_tensor(
                            tw[:, j - 1 : j + 1], cw[:, j - 1 : j + 1],
                            xTw[:, j - 1 : j + 1], AL.subtract,
                        )
                else:
                    nc.vector.tensor_tensor(tw[:, j], cw[:, j], xTw[:, j], AL.subtract)
                if ugroup:
                    if j == QNB - 1:
                        u_pend.append((gidx + 2, uw, rw, tw))
                else:
                    nc.vector.tensor_tensor(uw[:, j], rw[:, j], tw[:, j], AL.mult)
                if j == QNB - 1:
                    cw_prev[q] = cw
                    for ls in range(NLS):
                        pending.append((gidx, uw, q, ls, xts[ls], lc))
                drain_u(gidx)
                drain_pending(gidx)

         drain_u(1 << 30)
         while pending:
            drain_pending(1 << 30)

    nc.finalize()
    return nc


_NC_CACHE = None


def _get_nc():
    global _NC_CACHE
    if _NC_CACHE is None:
        _NC_CACHE = _build()
    return _NC_CACHE


def make_in_maps(x, W, b):
    # per-core layout (L, NB, D): every on-device access pattern is then
    # contiguous (strided PE moving-operand reads are ~4x slower on HW)
    return [
        dict(x=np.ascontiguousarray(x[:, i].transpose(0, 2, 1)), w=W, bb=b)
        for i in range(N_CORES)
    ]


def assemble(outs):
    # outs: per-core (L, NB, D) -> full (L, B, D, NB)
    return np.stack([o.transpose(0, 2, 1) for o in outs], axis=1)


def kernel(x: np.ndarray, W: np.ndarray, b: np.ndarray) -> np.ndarray:
    assert x.shape == (L, B, D, NB) and W.shape == (NB, D, 3 * D)
    from concourse.bass_utils import run_bass_kernel_spmd

    nc = _get_nc()
    x = np.asarray(x, dtype=np.float32)
    W = np.asarray(W, dtype=np.float32)
    b = np.asarray(b, dtype=np.float32)
    in_maps = make_in_maps(x, W, b)
    results = run_bass_kernel_spmd(nc, in_maps, core_ids=list(range(N_CORES))).results
    return assemble([results[i]["out"] for i in range(N_CORES)])


# revision 22
# speedup vs baseline: 1.2115x; 1.0019x over previous
"""BatchSRU Trainium2 kernel (nn_BatchSRU_27556510171508) — v5.

Full inputs: x (2048, 8, 128, 16) f32, W (16, 128, 384), b (16, 256).
Sharding: data-parallel over the inner batch B=8 -> one batch row per
NeuronCore (zero cross-core communication); W/b replicated.

v5 = v3 pipeline (all elementwise work on DVE; GpSimd measured ~3x
slower on HW and poisons the scan->highway chain) plus:
  - t = c - x^T computed as instance PAIRS ([128, 2*LC] ops) right
    after the pair's second scan — halves the op count and bubbles
  - u = r*t computed as ONE whole-group op [128, 4*LC], deferred two
    instances so it never sits between two scans on the in-order DVE
  - per-instance carry-save ACT copies eliminated: each chunk's scan
    chains initial= straight to the previous chunk's cw[:, j, -1:]
    (cw tags are per-group so the previous chunk's tile stays alive)
"""
import numpy as np
from contextlib import ExitStack

import concourse.bacc as bacc
import concourse.tile as tile
from concourse import mybir
from concourse.masks import make_identity

F32 = mybir.dt.float32
BF16 = mybir.dt.bfloat16
AL = mybir.AluOpType
AF = mybir.ActivationFunctionType

L, B, D, NB = 2048, 8, 128, 16
LC = 512
NCH = L // LC
QNB = 4
NQ = NB // QNB
NLS = LC // 128

N_CORES = 8


def _build(repeat: int = 1, unroll=False, carry_elim=True, ugroup=True, tpair=True, psplit=False, u_pool=False, t_pool=False):
    nc = bacc.Bacc("TRN2")
    x = nc.dram_tensor("x", [L, NB, D], F32, kind="ExternalInput")
    w = nc.dram_tensor("w", [NB, D, 3 * D], F32, kind="ExternalInput")
    bb = nc.dram_tensor("bb", [NB, 2 * D], F32, kind="ExternalInput")
    out = nc.dram_tensor("out", [L, NB, D], F32, kind="ExternalOutput")

    with tile.TileContext(nc) as tc, ExitStack() as ctx:
        const = ctx.enter_context(tc.tile_pool(name="const", bufs=1))

        ident = const.tile([128, 128], F32)
        make_identity(nc, ident)
        identb = const.tile([128, 128], BF16)
        make_identity(nc, identb)
        wr = const.tile([128, NB, 3 * D], BF16)
        bsb = const.tile([128, NB, 2], F32)
        nc.scalar.dma_start(out=bsb, in_=bb.rearrange("n (g d) -> d n g", d=128))
        carry = const.tile([128, NB], BF16)
        nc.vector.memset(carry, 0.0)

        with tc.tile_pool(name="wtmp_pool", bufs=1) as wtmp_pool:
            wtmp = wtmp_pool.tile([128, NB, 3 * D], F32)
            for wi in range(4):
                sl = slice(wi * 4, (wi + 1) * 4)
                nc.scalar.dma_start(out=wtmp[:, sl], in_=w.transpose([1, 0, 2])[:, sl])
                nc.vector.tensor_copy(wr[:, sl], wtmp[:, sl])

        xpool = ctx.enter_context(tc.tile_pool(name="xpool", bufs=2))
        sb = ctx.enter_context(tc.tile_pool(name="sb", bufs=2))
        if psplit:
            px = ctx.enter_context(tc.tile_pool(name="px", bufs=3, space="PSUM"))
            pfr = ctx.enter_context(tc.tile_pool(name="pfr", bufs=2, space="PSUM"))
            ph = ctx.enter_context(tc.tile_pool(name="ph", bufs=1, space="PSUM"))
        else:
            pu = ctx.enter_context(tc.tile_pool(name="pu", bufs=2, space="PSUM"))
            ph = ctx.enter_context(tc.tile_pool(name="ph", bufs=2, space="PSUM"))

        import contextlib

        pending = []
        u_pend = []
        dma_left = {}

        def emit_back_piece(rw, qq, ls, xts_t, plc):
            hps = ph.tile([128, QNB * 128], BF16, tag="ph", name="hps")
            for j in range(QNB):
                nc.tensor.transpose(
                    hps[:, j * 128 : (j + 1) * 128],
                    rw[:, j, ls * 128 : ls * 128 + 128],
                    identb,
                )
            xv = xts_t[:, qq * QNB * D : (qq + 1) * QNB * D]
            nc.vector.tensor_tensor(xv, hps, xv, AL.add)
            left, dlc, dls = dma_left[id(xts_t)]
            left -= 1
            dma_left[id(xts_t)] = (left, dlc, dls)
            if left == 0:
                l0 = dlc * LC + dls * 128
                nc.sync.dma_start(
                    out=out[l0 : l0 + 128].rearrange("l n d -> l (n d)"),
                    in_=xts_t,
                )

        def drain_pending(gidx, lag=6):
            if pending and gidx >= pending[0][0] + lag:
                ready, rw, qq, ls, xts_t, plc = pending.pop(0)
                emit_back_piece(rw, qq, ls, xts_t, plc)

        def drain_u(gidx):
            while u_pend and gidx >= u_pend[0][0]:
                _, uw_, rw_, tw_ = u_pend.pop(0)
                eng = nc.gpsimd if u_pool else nc.vector
                eng.tensor_tensor(uw_[:, :], rw_[:, :], tw_[:, :], AL.mult)

        cw_prev = {}
        n_unroll = repeat if unroll else 1
        loop_cm = (
            tc.For_i(0, repeat) if repeat > 1 and not unroll
            else contextlib.nullcontext()
        )
        with loop_cm:
         for lc0 in range(NCH * n_unroll):
            lc = lc0 % NCH
            xts = []
            for ls in range(NLS):
                xt_in = xpool.tile([128, D * NB], F32, tag=f"X{ls}")
                l0 = lc * LC + ls * 128
                nc.sync.dma_start(
                    out=xt_in, in_=x[l0 : l0 + 128].rearrange("l n d -> l (n d)")
                )
                xts.append(xt_in)
                dma_left[id(xt_in)] = (NQ, lc, ls)

            def gtiles(q):
                s = q % 2
                xTw = sb.tile([128, QNB, LC], BF16, tag=f"xT{s}", name=f"xT{s}")
                fw = sb.tile([128, QNB, LC], BF16, tag=f"f{s}", name=f"f{s}")
                rw = sb.tile([128, QNB, LC], BF16, tag=f"r{s}", name=f"r{s}")
                gw = sb.tile([128, QNB, LC], BF16, tag=f"g{s}", name=f"g{s}")
                cw = sb.tile(
                    [128, QNB, LC], BF16,
                    tag=(f"c{q}" if carry_elim else f"c{s}"), name="cw",
                )
                tw = sb.tile([128, QNB, LC], BF16, tag=f"t{s}", name=f"t{s}")
                uw = sb.tile([128, QNB, LC], BF16, tag=f"u{s}", name=f"u{s}")
                return xTw, fw, rw, gw, cw, tw, uw

            def in_transpose(i, pui, xTw):
                j = i % QNB
                px_i = pui[0]
                for ls in range(NLS):
                    xg = xts[ls][:, i * D : (i + 1) * D]
                    nc.tensor.transpose(
                        px_i[:, ls * 128 : (ls + 1) * 128], xg, ident
                    )
                nc.scalar.copy(xTw[:, j], px_i)

            grp = {}
            pu_i = [None] * (NB + 1)

            def alloc_pu():
                if psplit:
                    return (
                        px.tile([128, LC], F32, tag="px", name="px"),
                        pfr.tile([128, 2, LC], F32, tag="pfr", name="pfr"),
                    )
                t_ = pu.tile([128, 3, LC], F32, tag="pu", name="pu")
                return (t_[:, 0], t_[:, 1:3])

            grp[0] = gtiles(0)
            pu_i[0] = alloc_pu()
            in_transpose(0, pu_i[0], grp[0][0])

            for i in range(NB):
                q, j = i // QNB, i % QNB
                gidx = lc0 * NB + i
                xTw, fw, rw, gw, cw, tw, uw = grp[q]

                if i + 1 < NB:
                    qn = (i + 1) // QNB
                    if (i + 1) % QNB == 0:
                        grp[qn] = gtiles(qn)
                    pu_i[i + 1] = alloc_pu()
                    in_transpose(i + 1, pu_i[i + 1], grp[qn][0])

                px_i, pfr_i = pu_i[i]
                nc.tensor.matmul(
                    pfr_i[:, 0], wr[:, i, 128:256], xTw[:, j], start=True, stop=True
                )
                nc.tensor.matmul(
                    pfr_i[:, 1], wr[:, i, 256:384], xTw[:, j], start=True, stop=True
                )
                nc.tensor.matmul(
                    px_i, wr[:, i, 0:128], xTw[:, j], start=True, stop=True
                )
                nc.scalar.activation(
                    fw[:, j], pfr_i[:, 0], AF.Sigmoid, bias=bsb[:, i, 0:1], scale=1.0
                )
                nc.scalar.activation(
                    rw[:, j], pfr_i[:, 1], AF.Sigmoid, bias=bsb[:, i, 1:2], scale=1.0
                )
                nc.vector.scalar_tensor_tensor(
                    gw[:, j], fw[:, j], -1.0, px_i, AL.add, AL.mult
                )
                if carry_elim:
                    init = (
                        carry[:, i : i + 1]
                        if lc == 0
                        else cw_prev[q][:, j, LC - 1 : LC]
                    )
                else:
                    init = carry[:, i : i + 1]
                nc.vector.tensor_tensor_scan(
                    cw[:, j], fw[:, j], gw[:, j], init,
                    op0=AL.mult, op1=AL.subtract,
                )
                if not carry_elim:
                    nc.scalar.copy(carry[:, i : i + 1], cw[:, j, LC - 1 : LC])
                if t_pool:
                    nc.gpsimd.tensor_tensor(
                        tw[:, j], cw[:, j], xTw[:, j], AL.subtract
                    )
                elif tpair:
                    if j % 2 == 1:
                        nc.vector.tensor_tensor(
                            tw[:, j - 1 : j + 1], cw[:, j - 1 : j + 1],
                            xTw[:, j - 1 : j + 1], AL.subtract,
                        )
                else:
                    nc.vector.tensor_tensor(tw[:, j], cw[:, j], xTw[:, j], AL.subtract)
                if ugroup:
                    if j == QNB - 1:
                        u_pend.append((gidx + 2, uw, rw, tw))
                else:
                    nc.vector.tensor_tensor(uw[:, j], rw[:, j], tw[:, j], AL.mult)
                if j == QNB - 1:
                    cw_prev[q] = cw
                    for ls in range(NLS):
                        pending.append((gidx, uw, q, ls, xts[ls], lc))
                drain_u(gidx)
                drain_pending(gidx)

         drain_u(1 << 30)
         while pending:
            drain_pending(1 << 30)

    nc.finalize()
    return nc


_NC_CACHE = None


def _get_nc():
    global _NC_CACHE
    if _NC_CACHE is None:
        _NC_CACHE = _build()
    return _NC_CACHE


def make_in_maps(x, W, b):
    # per-core layout (L, NB, D): every on-device access pattern is then
    # contiguous (strided PE moving-operand reads are ~4x slower on HW)
    return [
        dict(x=np.ascontiguousarray(x[:, i].transpose(0, 2, 1)), w=W, bb=b)
        for i in range(N_CORES)
    ]


def assemble(outs):
    # outs: per-core (L, NB, D) -> full (L, B, D, NB)
    return np.stack([o.transpose(0, 2, 1) for o in outs], axis=1)


def kernel(x: np.ndarray, W: np.ndarray, b: np.ndarray) -> np.ndarray:
    assert x.shape == (L, B, D, NB) and W.shape == (NB, D, 3 * D)
    from concourse.bass_utils import run_bass_kernel_spmd

    nc = _get_nc()
    x = np.asarray(x, dtype=np.float32)
    W = np.asarray(W, dtype=np.float32)
    b = np.asarray(b, dtype=np.float32)
    in_maps = make_in_maps(x, W, b)
    results = run_bass_kernel_spmd(nc, in_maps, core_ids=list(range(N_CORES))).results
    return assemble([results[i]["out"] for i in range(N_CORES)])
